# revision 1
# baseline (speedup 1.0000x reference)
"""Trainium2 Bass kernel for a 2-layer dense GAT (nn_GAT_70446053589175).

kernel(**inputs) takes the FULL unsharded inputs (as produced by
setup_inputs) and returns the FULL [4096, 128] float32 output.

Sharding (8 NeuronCores, single SPMD program):
  Layer 1: 2 row-groups x 4 head-groups  (2048 rows, 2 heads per core)
  Layer 2: 8-way row split (512 rows per core)
  Exchange between layers via an in-kernel AllGather collective; the
  per-core s1' slice for layer 2 is extracted with a ReduceScatter.

Math: p_ij = exp(leakyrelu_0.2(s1_i + s2_j)) * adj_ij
          = max(exp(s1_i + s2_j), exp(0.2 s1_i) * exp(0.2 s2_j)) * adj_ij
  - exp(s1_i+s2_j) on ScalarE: Exp activation with per-partition bias (s2)
    over a broadcast s1 tile -> outer sum is free.
  - the second branch is rank-1: DVE scalar_tensor_tensor fuses
    (exp(0.2 s1)-tile * exp(0.2 s2)-scalar) max A in one op.
  - mask multiply as a bf16 tensor_tensor (adj pre-transposed/cast on host).
  - aggregation on PE with lhsT = [Wh | 1]: row D accumulates Z.
Layout is [j on partitions, i on free] throughout so the softmax-weighted
aggregation contracts over partitions.
"""
import sys
import os

for _p in ("/opt/trn_rl_repo", "/opt/pypackages",
           os.path.expanduser("~/.axon_site/_ro/trn_rl_repo"),
           os.path.expanduser("~/.axon_site/_ro/pypackages")):
    if os.path.isdir(_p) and _p not in sys.path:
        sys.path.insert(0, _p)

from contextlib import ExitStack

import numpy as np
import ml_dtypes

import concourse.bacc as bacc
import concourse.tile as tile
from concourse import mybir
from concourse.bass_utils import run_bass_kernel_spmd

dt = mybir.dt
AF = mybir.ActivationFunctionType
OP = mybir.AluOpType

BF16 = ml_dtypes.bfloat16
SLOPE = 0.2
_CACHE = {}


def _build(N=4096, F=512, D=64, H=8, O=128, n_cores=8, R=2, reps=1):
    HG = n_cores // R
    NHL = H // HG
    ROWS = N // R
    ROWS2 = N // n_cores
    JT = N // 128
    FT = F // 128
    IC = ROWS // 512
    IC2 = ROWS2 // 512
    DL = NHL * D
    HD = H * D
    OT = HD // 128
    JC = N // 512
    assert DL == 128 and OT == HG and O == 128

    nc = bacc.Bacc("TRN2", target_bir_lowering=False, debug=False, num_devices=n_cores)

    xT_in = nc.dram_tensor("xT", [F, N], dt.bfloat16, kind="ExternalInput").ap()
    xTown_in = nc.dram_tensor("xTown", [F, ROWS], dt.bfloat16, kind="ExternalInput").ap()
    adjT_in = nc.dram_tensor("adjT", [N, ROWS], dt.bfloat16, kind="ExternalInput").ap()
    adjT2_in = nc.dram_tensor("adjT2", [N, ROWS2], dt.bfloat16, kind="ExternalInput").ap()
    wloc_in = nc.dram_tensor("wloc", [F, DL], dt.bfloat16, kind="ExternalInput").ap()
    wtloc_in = nc.dram_tensor("wtloc", [DL, F], dt.bfloat16, kind="ExternalInput").ap()
    ablk_in = nc.dram_tensor("ablk", [DL, 2 * NHL], dt.bfloat16, kind="ExternalInput").ap()
    wout_in = nc.dram_tensor("wout", [HD, O], dt.bfloat16, kind="ExternalInput").ap()
    a2_in = nc.dram_tensor("a2", [O, 2], dt.bfloat16, kind="ExternalInput").ap()
    ones_in = nc.dram_tensor("ones", [1, 128], dt.float32, kind="ExternalInput").ap()
    ident_in = nc.dram_tensor("ident", [128, 128], dt.float32, kind="ExternalInput").ap()
    out_ext = nc.dram_tensor("out", [ROWS2, O], dt.float32, kind="ExternalOutput").ap()

    ht_shard = nc.dram_tensor("ht_shard", [DL, ROWS], dt.bfloat16)
    ht_all = nc.dram_tensor("ht_all", [n_cores * DL, ROWS], dt.bfloat16, addr_space="Shared")
    s1p_d = nc.dram_tensor("s1p_d", [1, N], dt.float32)
    s1p_rs = nc.dram_tensor("s1p_rs", [1, ROWS2], dt.float32)

    with tile.TileContext(nc) as tc, ExitStack() as top:
        cpool = top.enter_context(tc.tile_pool(name="const", bufs=1))
        ones_sb = cpool.tile([1, 128], dt.float32, tag="ones", name="ones")
        nc.sync.dma_start(ones_sb[:], ones_in)
        ident_sb = cpool.tile([128, 128], dt.float32, tag="ident", name="ident")
        nc.sync.dma_start(ident_sb[:], ident_in)
        identb_sb = cpool.tile([128, 128], dt.bfloat16, tag="identb", name="identb")
        nc.vector.tensor_copy(identb_sb[:], ident_sb[:])

        for _rep in range(reps):
            l2pf = ExitStack()
            with ExitStack() as l1s:
                l1pool = l1s.enter_context(tc.tile_pool(name="l1p", bufs=1))
                Wh_sb = l1pool.tile([128, JT * NHL * (D + 1)], dt.bfloat16, tag="whsb", name="whsb")
                S_sb = l1pool.tile([128, JT * 2 * NHL], dt.float32, tag="ssb", name="ssb")
                s1rep_sb = l1pool.tile([128, NHL * ROWS], dt.float32, tag="s1rep", name="s1rep")
                urep_sb = l1pool.tile([128, NHL * ROWS], dt.bfloat16, tag="urep", name="urep")
                vhat_sb = l1pool.tile([128, NHL * JT], dt.float32, tag="vhat", name="vhat")

                # ---- P1a: Wh (all j, local heads) + S = [s1|s2] per head ----
                with ExitStack() as p1:
                    xt_pool = p1.enter_context(tc.tile_pool(name="xt", bufs=1))
                    w_pool = p1.enter_context(tc.tile_pool(name="wp", bufs=1))
                    wt_stack = p1.enter_context(ExitStack())
                    ps_wt = wt_stack.enter_context(tc.tile_pool(name="pswt", bufs=1, space="PSUM"))

                    xT_sb = [xt_pool.tile([128, N], dt.bfloat16, tag=f"xt{ft}", name=f"xt{ft}")
                             for ft in range(FT)]
                    xTo_sb = [xt_pool.tile([128, ROWS], dt.bfloat16, tag=f"xto{ft}", name=f"xto{ft}")
                              for ft in range(FT)]
                    W_sb = [w_pool.tile([128, DL], dt.bfloat16, tag=f"wl{ft}", name=f"wl{ft}")
                            for ft in range(FT)]
                    for ft in range(FT):
                        nc.sync.dma_start(xT_sb[ft][:], xT_in[128 * ft:128 * (ft + 1), :])
                        nc.sync.dma_start(xTo_sb[ft][:], xTown_in[128 * ft:128 * (ft + 1), :])
                        nc.sync.dma_start(W_sb[ft][:], wloc_in[128 * ft:128 * (ft + 1), :])
                    WT_sb = w_pool.tile([DL, F], dt.bfloat16, tag="wtl", name="wtl")
                    nc.sync.dma_start(WT_sb[:], wtloc_in)
                    A_sb = w_pool.tile([DL, 2 * NHL], dt.bfloat16, tag="ablk", name="ablk")
                    nc.sync.dma_start(A_sb[:], ablk_in)

                    wt_ps = ps_wt.tile([128, FT * 2 * NHL], dt.float32, tag="wtps", name="wtps")
                    for ft in range(FT):
                        nc.tensor.matmul(wt_ps[:, ft * 2 * NHL:(ft + 1) * 2 * NHL],
                                         WT_sb[:, 128 * ft:128 * (ft + 1)], A_sb[:],
                                         start=True, stop=True)
                    Wt_sb = w_pool.tile([128, FT * 2 * NHL], dt.bfloat16, tag="wtsb", name="wtsb")
                    nc.vector.tensor_copy(Wt_sb[:], wt_ps[:])
                    wt_stack.close()

                    # ---- P1b first: own-row s1 + broadcast tiles (unblocks P3) ----
                    st_stack = p1.enter_context(ExitStack())
                    ps_st = st_stack.enter_context(tc.tile_pool(name="psst", bufs=1, space="PSUM"))
                    ps_rep = st_stack.enter_context(tc.tile_pool(name="psrep", bufs=2, space="PSUM"))
                    st_ps = ps_st.tile([2 * NHL, ROWS], dt.float32, tag="stps", name="stps")
                    for icx in range(IC):
                        for ft in range(FT):
                            nc.tensor.matmul(st_ps[:, 512 * icx:512 * (icx + 1)],
                                             Wt_sb[:, ft * 2 * NHL:(ft + 1) * 2 * NHL],
                                             xTo_sb[ft][:, 512 * icx:512 * (icx + 1)],
                                             start=(ft == 0), stop=(ft == FT - 1))
                    ST_sb = l1pool.tile([2 * NHL, ROWS], dt.float32, tag="stsb", name="stsb")
                    nc.vector.tensor_copy(ST_sb[:], st_ps[:])
                    s1row = [l1pool.tile([1, ROWS], dt.float32, tag=f"s1row{h}", name=f"s1row{h}")
                             for h in range(NHL)]
                    for h in range(NHL):
                        nc.sync.dma_start(s1row[h][:], ST_sb[2 * h:2 * h + 1, :])
                    for h in range(NHL):
                        for icx in range(IC):
                            rep_ps = ps_rep.tile([128, 512], dt.float32, tag="repps", name="repps")
                            nc.tensor.matmul(rep_ps[:], ones_sb[:],
                                             s1row[h][:, 512 * icx:512 * (icx + 1)],
                                             start=True, stop=True)
                            nc.scalar.copy(
                                s1rep_sb[:, h * ROWS + 512 * icx:h * ROWS + 512 * (icx + 1)],
                                rep_ps[:])
                    nc.scalar.activation(urep_sb[:], s1rep_sb[:], AF.Exp, scale=SLOPE)
                    st_stack.close()
                    ps_whs = p1.enter_context(tc.tile_pool(name="pswhs", bufs=2, space="PSUM"))

                    wh_view = Wh_sb[:].rearrange("p (j h d) -> p j h d", j=JT, h=NHL)
                    nc.vector.memset(wh_view[:, :, :, D:D + 1], 1.0)
                    for jt in range(JT):
                        wh_ps = ps_whs.tile([128, DL], dt.float32, tag="whps", name="whps")
                        s_ps = ps_whs.tile([128, 2 * NHL], dt.float32, tag="sps", name="sps")
                        for ft in range(FT):
                            lhs = xT_sb[ft][:, 128 * jt:128 * (jt + 1)]
                            nc.tensor.matmul(wh_ps[:], lhs, W_sb[ft][:],
                                             start=(ft == 0), stop=(ft == FT - 1))
                            nc.tensor.matmul(s_ps[:], lhs,
                                             Wt_sb[:, ft * 2 * NHL:(ft + 1) * 2 * NHL],
                                             start=(ft == 0), stop=(ft == FT - 1))
                        src = wh_ps[:].rearrange("p (h d) -> p h d", h=NHL)
                        nc.scalar.copy(wh_view[:, jt, :, 0:D], src)
                        nc.vector.tensor_copy(S_sb[:, jt * 2 * NHL:(jt + 1) * 2 * NHL], s_ps[:])
                        sl = S_sb[:].rearrange("p (j h two) -> p j h two", j=JT, h=NHL)
                        vl = vhat_sb[:].rearrange("p (h j) -> p h j", h=NHL)
                        for h in range(NHL):
                            nc.scalar.activation(
                                vl[:, h:h + 1, jt:jt + 1].rearrange("p one j -> p j one"),
                                sl[:, jt:jt + 1, h, 1:2], AF.Exp, scale=SLOPE)

                # ---- P3: layer-1 attention + aggregation ----
                with ExitStack() as p3:
                    mpool = p3.enter_context(tc.tile_pool(name="mt", bufs=6))
                    apool = p3.enter_context(tc.tile_pool(name="apl", bufs=4))
                    tpool = p3.enter_context(tc.tile_pool(name="tpl", bufs=4))
                    ppool = p3.enter_context(tc.tile_pool(name="ppl", bufs=4))
                    agg = p3.enter_context(tc.tile_pool(name="agg", bufs=1, space="PSUM"))
                    agg_ps = [[agg.tile([D + 1, 512], dt.float32, tag=f"agg{h}_{icx}",
                                        name=f"agg{h}_{icx}")
                               for icx in range(IC)] for h in range(NHL)]

                    for jt in range(JT):
                        mt = mpool.tile([128, ROWS], dt.bfloat16, tag="mt", name="mt")
                        nc.sync.dma_start(mt[:], adjT_in[128 * jt:128 * (jt + 1), :])
                        for h in range(NHL):
                            a_t = apool.tile([128, ROWS], dt.bfloat16, tag="a", name="a")
                            nc.scalar.activation(
                                a_t[:], s1rep_sb[:, h * ROWS:(h + 1) * ROWS], AF.Exp,
                                bias=S_sb[:, jt * 2 * NHL + 2 * h + 1:jt * 2 * NHL + 2 * h + 2],
                                scale=1.0)
                            t_t = tpool.tile([128, ROWS], dt.bfloat16, tag="t", name="t")
                            nc.vector.scalar_tensor_tensor(
                                t_t[:], urep_sb[:, h * ROWS:(h + 1) * ROWS],
                                vhat_sb[:, h * JT + jt:h * JT + jt + 1], a_t[:],
                                op0=OP.mult, op1=OP.max)
                            p_t = ppool.tile([128, ROWS], dt.bfloat16, tag="p", name="p")
                            nc.vector.tensor_tensor(p_t[:], t_t[:], mt[:], OP.mult)
                            lhs = Wh_sb[:, (jt * NHL + h) * (D + 1):(jt * NHL + h + 1) * (D + 1)]
                            for icx in range(IC):
                                nc.tensor.matmul(agg_ps[h][icx][:], lhs,
                                                 p_t[:, 512 * icx:512 * (icx + 1)],
                                                 start=(jt == 0), stop=(jt == JT - 1))

                    evac = p3.enter_context(tc.tile_pool(name="evac", bufs=1))
                    raw_sb = evac.tile([D, NHL * ROWS], dt.bfloat16, tag="raw", name="raw")
                    zc_sb = evac.tile([D + 1, NHL * IC * 512], dt.float32, tag="zc", name="zc")
                    for h in range(NHL):
                        for icx in range(IC):
                            o = (h * IC + icx) * 512
                            nc.vector.tensor_copy(raw_sb[:, o:o + 512], agg_ps[h][icx][0:D, :])
                            nc.scalar.copy(zc_sb[D:D + 1, o:o + 512], agg_ps[h][icx][D:D + 1, :])
                    rc_sb = evac.tile([D + 1, NHL * IC * 512], dt.float32, tag="rc", name="rc")
                    nc.vector.reciprocal(rc_sb[D:D + 1, :], zc_sb[D:D + 1, :])
                    rrow = evac.tile([1, NHL * IC * 512], dt.float32, tag="rrow", name="rrow")
                    nc.sync.dma_start(rrow[:], rc_sb[D:D + 1, :])

                # ---- P4: h = elu(raw / Z) ----
                with ExitStack() as p4:
                    ps_rr = p4.enter_context(tc.tile_pool(name="psrr", bufs=2, space="PSUM"))
                    n4 = p4.enter_context(tc.tile_pool(name="n4", bufs=1))
                    rrep_sb = n4.tile([D, NHL * ROWS], dt.bfloat16, tag="rrep", name="rrep")
                    for k in range(NHL * IC):
                        rr_ps = ps_rr.tile([D, 512], dt.float32, tag="rrps", name="rrps")
                        nc.tensor.matmul(rr_ps[:], ones_sb[:, 0:D],
                                         rrow[:, 512 * k:512 * (k + 1)], start=True, stop=True)
                        nc.scalar.copy(rrep_sb[:, 512 * k:512 * (k + 1)], rr_ps[:])
                    t_n = n4.tile([D, NHL * ROWS], dt.bfloat16, tag="tn", name="tn")
                    nc.vector.tensor_tensor(t_n[:], raw_sb[:], rrep_sb[:], OP.mult)
                    m0 = n4.tile([D, NHL * ROWS], dt.bfloat16, tag="m0", name="m0")
                    nc.vector.tensor_scalar(m0[:], t_n[:], 0.0, None, op0=OP.min)
                    e0 = n4.tile([D, NHL * ROWS], dt.bfloat16, tag="e0", name="e0")
                    nc.scalar.activation(e0[:], m0[:], AF.Exp)
                    d0 = n4.tile([D, NHL * ROWS], dt.bfloat16, tag="d0", name="d0")
                    nc.vector.tensor_tensor(d0[:], t_n[:], m0[:], OP.subtract)
                    elu_sb = n4.tile([D, NHL * ROWS], dt.bfloat16, tag="elu", name="elu")
                    nc.vector.scalar_tensor_tensor(elu_sb[:], e0[:], -1.0, d0[:],
                                                   op0=OP.add, op1=OP.add)
                    for h in range(NHL):
                        for icx in range(IC):
                            o = (h * IC + icx) * 512
                            nc.sync.dma_start(
                                ht_shard.ap()[D * h:D * (h + 1), 512 * icx:512 * (icx + 1)],
                                elu_sb[:, o:o + 512])

            # ---- P5: exchange (adjT2 prefetch overlaps collective + L2 prep) ----
            a2pf = l2pf.enter_context(tc.tile_pool(name="a2pf", bufs=1))
            adjT2_sb = a2pf.tile([128, 32 * 512], dt.bfloat16, tag="adjt2sb", name="adjt2sb")
            for _jt in range(32):
                nc.sync.dma_start(adjT2_sb[:, 512 * _jt:512 * (_jt + 1)],
                                  adjT2_in[128 * _jt:128 * (_jt + 1), :])
            nc.gpsimd.collective_compute(
                "AllGather", OP.bypass, replica_groups=[list(range(n_cores))],
                ins=[ht_shard.ap()], outs=[ht_all.ap()])

            # ---- P6: layer-2 prep ----
            with ExitStack() as p6:
                htp = p6.enter_context(tc.tile_pool(name="htp", bufs=1))
                l2p = p6.enter_context(tc.tile_pool(name="l2p", bufs=1))
                HT_sb = [htp.tile([128, N], dt.bfloat16, tag=f"ht{g}", name=f"ht{g}")
                         for g in range(OT)]
                for g in range(OT):
                    for rr in range(R):
                        core_idx = rr * HG + g
                        nc.sync.dma_start(HT_sb[g][:, ROWS * rr:ROWS * (rr + 1)],
                                          ht_all.ap()[DL * core_idx:DL * (core_idx + 1), :])
                Wo_sb = [l2p.tile([128, O], dt.bfloat16, tag=f"wo{g}", name=f"wo{g}")
                         for g in range(OT)]
                for g in range(OT):
                    nc.sync.dma_start(Wo_sb[g][:], wout_in[128 * g:128 * (g + 1), :])
                A2_sb = l2p.tile([O, 2], dt.bfloat16, tag="a2", name="a2")
                nc.sync.dma_start(A2_sb[:], a2_in)

                with ExitStack() as p6a:
                    p6w = p6a.enter_context(ExitStack())
                    ps_w2 = p6w.enter_context(tc.tile_pool(name="psw2", bufs=2, space="PSUM"))
                    ps_s2 = p6w.enter_context(tc.tile_pool(name="pss2", bufs=2, space="PSUM"))
                    Wh2T_sb = l2p.tile([O, N], dt.bfloat16, tag="wh2t", name="wh2t")
                    S2_sb = l2p.tile([2, N], dt.float32, tag="s2sb", name="s2sb")
                    for jc in range(JC):
                        w2_ps = ps_w2.tile([O, 512], dt.float32, tag="w2ps", name="w2ps")
                        for g in range(OT):
                            nc.tensor.matmul(w2_ps[:], Wo_sb[g][:],
                                             HT_sb[g][:, 512 * jc:512 * (jc + 1)],
                                             start=(g == 0), stop=(g == OT - 1))
                        nc.scalar.copy(Wh2T_sb[:, 512 * jc:512 * (jc + 1)], w2_ps[:])
                        s2_ps = ps_s2.tile([2, 512], dt.float32, tag="s2ps", name="s2ps")
                        nc.tensor.matmul(s2_ps[:], A2_sb[:],
                                         Wh2T_sb[:, 512 * jc:512 * (jc + 1)],
                                         start=True, stop=True)
                        nc.scalar.copy(S2_sb[:, 512 * jc:512 * (jc + 1)], s2_ps[:])
                    nc.sync.dma_start(s1p_d.ap()[0:1, :], S2_sb[0:1, :])
                    nc.gpsimd.collective_compute(
                        "ReduceScatter", OP.add, replica_groups=[list(range(n_cores))],
                        ins=[s1p_d.ap()], outs=[s1p_rs.ap()])

                    p6w.close()
                    ps_t = p6a.enter_context(tc.tile_pool(name="pst", bufs=1, space="PSUM"))
                    ps_tr = p6a.enter_context(tc.tile_pool(name="pstr", bufs=2, space="PSUM"))
                    s2t_ps = ps_t.tile([128, 2 * JT], dt.float32, tag="s2tps", name="s2tps")
                    for jt in range(JT):
                        nc.tensor.matmul(s2t_ps[:, 2 * jt:2 * (jt + 1)],
                                         S2_sb[:, 128 * jt:128 * (jt + 1)],
                                         ident_sb[0:2, 0:2], is_transpose=True,
                                         start=True, stop=True)
                    S2T_sb = l2p.tile([128, 2 * JT], dt.float32, tag="s2tsb", name="s2tsb")
                    nc.vector.tensor_copy(S2T_sb[:], s2t_ps[:])

                    Wh2o_sb = l2p.tile([128, JT * (O + 2)], dt.bfloat16, tag="wh2o", name="wh2o")
                    w2o_view = Wh2o_sb[:].rearrange("p (j c) -> p j c", j=JT)
                    for jt in range(JT):
                        tr_ps = ps_tr.tile([128, 128], dt.bfloat16, tag="trps", name="trps")
                        nc.tensor.matmul(tr_ps[:], Wh2T_sb[:, 128 * jt:128 * (jt + 1)],
                                         identb_sb[:], is_transpose=True,
                                         start=True, stop=True)
                        nc.scalar.copy(w2o_view[:, jt, 0:O], tr_ps[:])
                    nc.vector.memset(w2o_view[:, :, O:O + 1], 1.0)

                    ps_r2 = p6a.enter_context(tc.tile_pool(name="psr2", bufs=2, space="PSUM"))
                    s1row2 = l2p.tile([1, ROWS2], dt.float32, tag="s1row2", name="s1row2")
                    nc.sync.dma_start(s1row2[:], s1p_rs.ap())
                    s1rep2_sb = l2p.tile([128, ROWS2], dt.float32, tag="s1rep2", name="s1rep2")
                    for icx in range(IC2):
                        r2_ps = ps_r2.tile([128, 512], dt.float32, tag="r2ps", name="r2ps")
                        nc.tensor.matmul(r2_ps[:], ones_sb[:],
                                         s1row2[:, 512 * icx:512 * (icx + 1)],
                                         start=True, stop=True)
                        nc.scalar.activation(s1rep2_sb[:, 512 * icx:512 * (icx + 1)], r2_ps[:],
                                             AF.Identity, scale=1.0 / n_cores)
                    u2rep_sb = l2p.tile([128, ROWS2], dt.bfloat16, tag="u2rep", name="u2rep")
                    nc.scalar.activation(u2rep_sb[:], s1rep2_sb[:], AF.Exp, scale=SLOPE)
                    v2hat_sb = l2p.tile([128, JT], dt.float32, tag="v2hat", name="v2hat")
                    s2tv = S2T_sb[:].rearrange("p (j two) -> p j two", j=JT)
                    nc.scalar.activation(
                        v2hat_sb[:].rearrange("p (j one) -> p j one", one=1),
                        s2tv[:, :, 1:2], AF.Exp, scale=SLOPE)

                # ---- P7: layer-2 attention ----
                with ExitStack() as p7:
                    m2p = p7.enter_context(tc.tile_pool(name="m2", bufs=6))
                    a2p = p7.enter_context(tc.tile_pool(name="a2p", bufs=3))
                    t2p = p7.enter_context(tc.tile_pool(name="t2p", bufs=3))
                    p2p = p7.enter_context(tc.tile_pool(name="p2p", bufs=3))
                    agg2 = p7.enter_context(tc.tile_pool(name="agg2", bufs=1, space="PSUM"))
                    o_ps = [agg2.tile([128, O + 1], dt.float32, tag=f"ops{it}", name=f"ops{it}")
                            for it in range(ROWS2 // 128)]
                    for jt in range(JT):
                        mt2 = adjT2_sb[:, 512 * jt:512 * (jt + 1)]
                        a2t = a2p.tile([128, ROWS2], dt.bfloat16, tag="a2t", name="a2t")
                        nc.scalar.activation(a2t[:], s1rep2_sb[:], AF.Exp,
                                             bias=S2T_sb[:, 2 * jt + 1:2 * jt + 2], scale=1.0)
                        t2t = t2p.tile([128, ROWS2], dt.bfloat16, tag="t2t", name="t2t")
                        nc.vector.scalar_tensor_tensor(t2t[:], u2rep_sb[:],
                                                       v2hat_sb[:, jt:jt + 1], a2t[:],
                                                       op0=OP.mult, op1=OP.max)
                        p2t = p2p.tile([128, ROWS2], dt.bfloat16, tag="p2t", name="p2t")
                        nc.vector.tensor_tensor(p2t[:], t2t[:], mt2, OP.mult)
                        for it in range(ROWS2 // 128):
                            nc.tensor.matmul(o_ps[it][:], p2t[:, 128 * it:128 * (it + 1)],
                                             w2o_view[:, jt, 0:O + 1],
                                             start=(jt == 0), stop=(jt == JT - 1))
                    fo = p7.enter_context(tc.tile_pool(name="fo", bufs=4))
                    for it in range(ROWS2 // 128):
                        r2v = fo.tile([128, 1], dt.float32, tag="r2v", name="r2v")
                        nc.vector.reciprocal(r2v[:], o_ps[it][:, O:O + 1])
                        ot = fo.tile([128, O], dt.float32, tag="ot", name="ot")
                        nc.vector.tensor_scalar(ot[:], o_ps[it][:, 0:O], r2v[:, 0:1], None,
                                                op0=OP.mult)
                        if reps == 1:
                            nc.sync.dma_start(out_ext[128 * it:128 * (it + 1), :], ot[:])
                        else:
                            # timing builds: accumulate so repeated bodies stay live
                            nc.gpsimd.dma_start(out_ext[128 * it:128 * (it + 1), :],
                                                ot[:], accum_op=OP.add)
            l2pf.close()

    nc.compile()
    return nc


def _make_in_maps(x, adj, W_heads, a1_heads, a2_heads, W_out, a1_out, a2_out,
                  n_cores=8, R=2):
    N, F = x.shape
    H, _, D = W_heads.shape
    HG = n_cores // R
    NHL = H // HG
    ROWS = N // R
    ROWS2 = N // n_cores

    xT = np.ascontiguousarray(x.T).astype(BF16)
    adjT = np.ascontiguousarray(adj.T).astype(BF16)
    ones = np.ones((1, 128), np.float32)
    ident = np.eye(128, dtype=np.float32)
    wout = np.ascontiguousarray(W_out).astype(BF16)
    a2m = np.stack([a1_out, a2_out], axis=1).astype(BF16)

    in_maps = []
    for c in range(n_cores):
        rr, hg = c // HG, c % HG
        heads = list(range(NHL * hg, NHL * (hg + 1)))
        r0 = ROWS * rr
        wloc = np.concatenate([W_heads[h] for h in heads], axis=1).astype(BF16)
        wtloc = np.concatenate([W_heads[h].T for h in heads], axis=0).astype(BF16)
        ablk = np.zeros((NHL * D, 2 * NHL), np.float32)
        for k, h in enumerate(heads):
            ablk[k * D:(k + 1) * D, 2 * k] = a1_heads[h]
            ablk[k * D:(k + 1) * D, 2 * k + 1] = a2_heads[h]
        in_maps.append({
            "xT": xT,
            "xTown": np.ascontiguousarray(xT[:, r0:r0 + ROWS]),
            "adjT": np.ascontiguousarray(adjT[:, r0:r0 + ROWS]),
            "adjT2": np.ascontiguousarray(adjT[:, ROWS2 * c:ROWS2 * (c + 1)]),
            "wloc": wloc,
            "wtloc": wtloc,
            "ablk": ablk.astype(BF16),
            "wout": wout,
            "a2": a2m,
            "ones": ones,
            "ident": ident,
        })
    return in_maps


def kernel(x, adj, W_heads, a1_heads, a2_heads, W_out, a1_out, a2_out):
    x = np.asarray(x, dtype=np.float32)
    adj = np.asarray(adj)
    W_heads = np.asarray(W_heads, dtype=np.float32)
    a1_heads = np.asarray(a1_heads, dtype=np.float32)
    a2_heads = np.asarray(a2_heads, dtype=np.float32)
    W_out = np.asarray(W_out, dtype=np.float32)
    a1_out = np.asarray(a1_out, dtype=np.float32)
    a2_out = np.asarray(a2_out, dtype=np.float32)

    if "nc" not in _CACHE:
        _CACHE["nc"] = _build()
    nc = _CACHE["nc"]
    in_maps = _make_in_maps(x, adj, W_heads, a1_heads, a2_heads,
                            W_out, a1_out, a2_out)
    res = run_bass_kernel_spmd(nc, in_maps, list(range(8)))
    out = np.concatenate([r["out"] for r in res.results], axis=0)
    return out.astype(np.float32)


if __name__ == "__main__":
    import jax
    key = jax.random.key(0)
    ks = jax.random.split(key, 8)
    import jax.numpy as jnp
    N, F, D, H, O = 4096, 512, 64, 8, 128
    ins = {
        "x": np.asarray(jax.random.normal(ks[0], (N, F), dtype=jnp.float32)),
        "adj": np.asarray(jax.random.randint(ks[1], (N, N), 0, 2, dtype=jnp.int32)),
        "W_heads": np.asarray(jax.random.normal(ks[2], (H, F, D), dtype=jnp.float32) * 0.05),
        "a1_heads": np.asarray(jax.random.normal(ks[3], (H, D), dtype=jnp.float32) * 0.05),
        "a2_heads": np.asarray(jax.random.normal(ks[4], (H, D), dtype=jnp.float32) * 0.05),
        "W_out": np.asarray(jax.random.normal(ks[5], (H * D, O), dtype=jnp.float32) * 0.05),
        "a1_out": np.asarray(jax.random.normal(ks[6], (O,), dtype=jnp.float32) * 0.05),
        "a2_out": np.asarray(jax.random.normal(ks[7], (O,), dtype=jnp.float32) * 0.05),
    }
    out = kernel(**ins)
    print("out", out.shape, out.dtype, float(np.abs(out).max()))



# revision 5
# speedup vs baseline: 1.3108x; 1.3108x over previous
"""Trainium2 Bass kernel for a 2-layer dense GAT (nn_GAT_70446053589175).

kernel(**inputs) takes the FULL unsharded inputs (as produced by
setup_inputs) and returns the FULL [4096, 128] float32 output.

Sharding (8 NeuronCores, single SPMD program):
  Layer 1: 2 row-groups x 4 head-groups  (2048 rows, 2 heads per core)
  Layer 2: 8-way row split (512 rows per core)
  Exchange between layers via an in-kernel AllGather collective; the
  per-core s1' slice for layer 2 is extracted with a ReduceScatter.

Math (row-rescaled softmax numerator):
  softmax rows are invariant to per-row (i) positive scaling, so divide
  the numerator exp(leakyrelu_0.2(s1_i + s2_j)) by exp(0.2*s1_i):
    p'_ij = m_ij * max( e^{0.8 s1_i} * e^{s2_j},  e^{0.2 s2_j} )
  The leaky branch is now constant in i, so per (jt, head) tile:
    t  = tensor_scalar(w1rep, w2_j, v2_j, mult, max)   # DVE 4x mode
    p' = tensor_tensor(t, mask, mult)                  # DVE 2x mode
  (no per-tile ScalarE op, no 1x scalar_tensor_tensor).
  Aggregation on PE with lhsT = [Wh | 1]: row D accumulates Z.
  Layer-1 output is written as elu(h)+1 = e^{min(x,0)} + relu(x); the
  +1 shift is removed in layer 2 by subtracting colsum(W_out) from Wh2
  (softmax rows sum to 1), applied for free via the activation bias
  during PSUM evacuation.
Layout is [j on partitions, i on free] throughout so the softmax-weighted
aggregation contracts over partitions.
"""
import sys
import os

for _p in ("/opt/trn_rl_repo", "/opt/pypackages",
           os.path.expanduser("~/.axon_site/_ro/trn_rl_repo"),
           os.path.expanduser("~/.axon_site/_ro/pypackages")):
    if os.path.isdir(_p) and _p not in sys.path:
        sys.path.insert(0, _p)

from contextlib import ExitStack

import numpy as np
import ml_dtypes

import concourse.bacc as bacc
import concourse.tile as tile
from concourse import mybir
from concourse.bass_utils import run_bass_kernel_spmd

dt = mybir.dt
AF = mybir.ActivationFunctionType
OP = mybir.AluOpType

BF16 = ml_dtypes.bfloat16
SLOPE = 0.2
_CACHE = {}


def _build(N=4096, F=512, D=64, H=8, O=128, n_cores=8, R=2, reps=1):
    HG = n_cores // R
    NHL = H // HG
    ROWS = N // R
    ROWS2 = N // n_cores
    JT = N // 128
    FT = F // 128
    IC = ROWS // 512
    IC2 = ROWS2 // 512
    DL = NHL * D
    HD = H * D
    OT = HD // 128
    JC = N // 512
    assert DL == 128 and OT == HG and O == 128

    nc = bacc.Bacc("TRN2", target_bir_lowering=False, debug=False, num_devices=n_cores)

    xT_in = nc.dram_tensor("xT", [F, N], dt.bfloat16, kind="ExternalInput").ap()
    xTown_in = nc.dram_tensor("xTown", [F, ROWS], dt.bfloat16, kind="ExternalInput").ap()
    adjT_in = nc.dram_tensor("adjT", [N, ROWS], dt.bfloat16, kind="ExternalInput").ap()
    adjT2_in = nc.dram_tensor("adjT2", [N, ROWS2], dt.bfloat16, kind="ExternalInput").ap()
    wloc_in = nc.dram_tensor("wloc", [F, DL], dt.bfloat16, kind="ExternalInput").ap()
    wtloc_in = nc.dram_tensor("wtloc", [DL, F], dt.bfloat16, kind="ExternalInput").ap()
    ablk_in = nc.dram_tensor("ablk", [DL, 2 * NHL], dt.bfloat16, kind="ExternalInput").ap()
    wout_in = nc.dram_tensor("wout", [HD, O], dt.bfloat16, kind="ExternalInput").ap()
    a2_in = nc.dram_tensor("a2", [O, 2], dt.bfloat16, kind="ExternalInput").ap()
    ncs_in = nc.dram_tensor("ncs", [O, 1], dt.float32, kind="ExternalInput").ap()
    ones_in = nc.dram_tensor("ones", [1, 128], dt.float32, kind="ExternalInput").ap()
    ident_in = nc.dram_tensor("ident", [128, 128], dt.float32, kind="ExternalInput").ap()
    out_ext = nc.dram_tensor("out", [ROWS2, O], dt.float32, kind="ExternalOutput").ap()

    ht_shard = nc.dram_tensor("ht_shard", [DL, ROWS], dt.bfloat16)
    ht_all = nc.dram_tensor("ht_all", [n_cores * DL, ROWS], dt.bfloat16, addr_space="Shared")
    s1p_d = nc.dram_tensor("s1p_d", [1, N], dt.float32)
    s1p_rs = nc.dram_tensor("s1p_rs", [1, ROWS2], dt.float32)

    with tile.TileContext(nc) as tc, ExitStack() as top:
        cpool = top.enter_context(tc.tile_pool(name="const", bufs=1))
        ones_sb = cpool.tile([1, 128], dt.float32, tag="ones", name="ones")
        nc.sync.dma_start(ones_sb[:], ones_in)
        ident_sb = cpool.tile([128, 128], dt.float32, tag="ident", name="ident")
        nc.sync.dma_start(ident_sb[:], ident_in)
        identb_sb = cpool.tile([128, 128], dt.bfloat16, tag="identb", name="identb")
        nc.vector.tensor_copy(identb_sb[:], ident_sb[:])

        for _rep in range(reps):
            l2pf = ExitStack()
            with ExitStack() as l1s:
                l1pool = l1s.enter_context(tc.tile_pool(name="l1p", bufs=1))
                Wh_sb = l1pool.tile([128, JT * NHL * (D + 1)], dt.bfloat16, tag="whsb", name="whsb")
                S_sb = l1pool.tile([128, JT * 2 * NHL], dt.float32, tag="ssb", name="ssb")
                w1rep_sb = l1pool.tile([128, NHL * ROWS], dt.bfloat16, tag="w1rep", name="w1rep")
                W2_sb = l1pool.tile([128, NHL * JT], dt.float32, tag="w2sb", name="w2sb")
                V2_sb = l1pool.tile([128, NHL * JT], dt.float32, tag="v2sb", name="v2sb")

                # ---- P1a: Wh (all j, local heads) + S = [s1|s2] per head ----
                with ExitStack() as p1:
                    xt_pool = p1.enter_context(tc.tile_pool(name="xt", bufs=1))
                    w_pool = p1.enter_context(tc.tile_pool(name="wp", bufs=1))
                    wt_stack = p1.enter_context(ExitStack())
                    ps_wt = wt_stack.enter_context(tc.tile_pool(name="pswt", bufs=1, space="PSUM"))

                    xT_sb = [xt_pool.tile([128, N], dt.bfloat16, tag=f"xt{ft}", name=f"xt{ft}")
                             for ft in range(FT)]
                    xTo_sb = [xt_pool.tile([128, ROWS], dt.bfloat16, tag=f"xto{ft}", name=f"xto{ft}")
                              for ft in range(FT)]
                    W_sb = [w_pool.tile([128, DL], dt.bfloat16, tag=f"wl{ft}", name=f"wl{ft}")
                            for ft in range(FT)]
                    for ft in range(FT):
                        nc.sync.dma_start(xT_sb[ft][:], xT_in[128 * ft:128 * (ft + 1), :])
                        nc.sync.dma_start(xTo_sb[ft][:], xTown_in[128 * ft:128 * (ft + 1), :])
                        nc.sync.dma_start(W_sb[ft][:], wloc_in[128 * ft:128 * (ft + 1), :])
                    WT_sb = w_pool.tile([DL, F], dt.bfloat16, tag="wtl", name="wtl")
                    nc.sync.dma_start(WT_sb[:], wtloc_in)
                    A_sb = w_pool.tile([DL, 2 * NHL], dt.bfloat16, tag="ablk", name="ablk")
                    nc.sync.dma_start(A_sb[:], ablk_in)

                    wt_ps = ps_wt.tile([128, FT * 2 * NHL], dt.float32, tag="wtps", name="wtps")
                    for ft in range(FT):
                        nc.tensor.matmul(wt_ps[:, ft * 2 * NHL:(ft + 1) * 2 * NHL],
                                         WT_sb[:, 128 * ft:128 * (ft + 1)], A_sb[:],
                                         start=True, stop=True)
                    Wt_sb = w_pool.tile([128, FT * 2 * NHL], dt.bfloat16, tag="wtsb", name="wtsb")
                    nc.vector.tensor_copy(Wt_sb[:], wt_ps[:])
                    wt_stack.close()

                    # ---- P1b first: own-row s1 -> w1rep = exp(0.8 s1) ----
                    st_stack = p1.enter_context(ExitStack())
                    ps_st = st_stack.enter_context(tc.tile_pool(name="psst", bufs=1, space="PSUM"))
                    ps_rep = st_stack.enter_context(tc.tile_pool(name="psrep", bufs=2, space="PSUM"))
                    st_ps = ps_st.tile([2 * NHL, ROWS], dt.float32, tag="stps", name="stps")
                    for icx in range(IC):
                        for ft in range(FT):
                            nc.tensor.matmul(st_ps[:, 512 * icx:512 * (icx + 1)],
                                             Wt_sb[:, ft * 2 * NHL:(ft + 1) * 2 * NHL],
                                             xTo_sb[ft][:, 512 * icx:512 * (icx + 1)],
                                             start=(ft == 0), stop=(ft == FT - 1))
                    ST_sb = l1pool.tile([2 * NHL, ROWS], dt.float32, tag="stsb", name="stsb")
                    nc.vector.tensor_copy(ST_sb[:], st_ps[:])
                    s1row = [l1pool.tile([1, ROWS], dt.float32, tag=f"s1row{h}", name=f"s1row{h}")
                             for h in range(NHL)]
                    for h in range(NHL):
                        nc.sync.dma_start(s1row[h][:], ST_sb[2 * h:2 * h + 1, :])
                    for h in range(NHL):
                        for icx in range(IC):
                            rep_ps = ps_rep.tile([128, 512], dt.float32, tag="repps", name="repps")
                            nc.tensor.matmul(rep_ps[:], ones_sb[:],
                                             s1row[h][:, 512 * icx:512 * (icx + 1)],
                                             start=True, stop=True)
                            nc.scalar.activation(
                                w1rep_sb[:, h * ROWS + 512 * icx:h * ROWS + 512 * (icx + 1)],
                                rep_ps[:], AF.Exp, scale=1.0 - SLOPE)
                    st_stack.close()
                    ps_whs = p1.enter_context(tc.tile_pool(name="pswhs", bufs=2, space="PSUM"))

                    wh_view = Wh_sb[:].rearrange("p (j h d) -> p j h d", j=JT, h=NHL)
                    nc.vector.memset(wh_view[:, :, :, D:D + 1], 1.0)
                    for jt in range(JT):
                        wh_ps = ps_whs.tile([128, DL], dt.float32, tag="whps", name="whps")
                        s_ps = ps_whs.tile([128, 2 * NHL], dt.float32, tag="sps", name="sps")
                        for ft in range(FT):
                            lhs = xT_sb[ft][:, 128 * jt:128 * (jt + 1)]
                            nc.tensor.matmul(wh_ps[:], lhs, W_sb[ft][:],
                                             start=(ft == 0), stop=(ft == FT - 1))
                            nc.tensor.matmul(s_ps[:], lhs,
                                             Wt_sb[:, ft * 2 * NHL:(ft + 1) * 2 * NHL],
                                             start=(ft == 0), stop=(ft == FT - 1))
                        src = wh_ps[:].rearrange("p (h d) -> p h d", h=NHL)
                        nc.scalar.copy(wh_view[:, jt, :, 0:D], src)
                        nc.vector.tensor_copy(S_sb[:, jt * 2 * NHL:(jt + 1) * 2 * NHL], s_ps[:])
                    # batched per-(h,jt) scalars: w2 = e^{s2}, v2 = e^{0.2 s2}
                    sl = S_sb[:].rearrange("p (j h two) -> p h j two", j=JT, h=NHL)
                    w2v = W2_sb[:].rearrange("p (h j) -> p h j", h=NHL)
                    v2v = V2_sb[:].rearrange("p (h j) -> p h j", h=NHL)
                    for h in range(NHL):
                        nc.scalar.activation(
                            w2v[:, h:h + 1, :].rearrange("p one j -> p j one"),
                            sl[:, h, :, 1:2], AF.Exp, scale=1.0)
                        nc.scalar.activation(
                            v2v[:, h:h + 1, :].rearrange("p one j -> p j one"),
                            sl[:, h, :, 1:2], AF.Exp, scale=SLOPE)

                # ---- P3: layer-1 attention + aggregation ----
                with ExitStack() as p3:
                    mpool = p3.enter_context(tc.tile_pool(name="mt", bufs=6))
                    tpool = p3.enter_context(tc.tile_pool(name="tpl", bufs=4))
                    ppool = p3.enter_context(tc.tile_pool(name="ppl", bufs=4))
                    agg = p3.enter_context(tc.tile_pool(name="agg", bufs=1, space="PSUM"))
                    agg_ps = [[agg.tile([D + 1, 512], dt.float32, tag=f"agg{h}_{icx}",
                                        name=f"agg{h}_{icx}")
                               for icx in range(IC)] for h in range(NHL)]

                    for jt in range(JT):
                        mt = mpool.tile([128, ROWS], dt.bfloat16, tag="mt", name="mt")
                        nc.sync.dma_start(mt[:], adjT_in[128 * jt:128 * (jt + 1), :])
                        for h in range(NHL):
                            t_t = tpool.tile([128, ROWS], dt.bfloat16, tag="t", name="t")
                            nc.vector.tensor_scalar(
                                t_t[:], w1rep_sb[:, h * ROWS:(h + 1) * ROWS],
                                W2_sb[:, h * JT + jt:h * JT + jt + 1],
                                V2_sb[:, h * JT + jt:h * JT + jt + 1],
                                op0=OP.mult, op1=OP.max)
                            p_t = ppool.tile([128, ROWS], dt.bfloat16, tag="p", name="p")
                            nc.vector.tensor_tensor(p_t[:], t_t[:], mt[:], OP.mult)
                            lhs = Wh_sb[:, (jt * NHL + h) * (D + 1):(jt * NHL + h + 1) * (D + 1)]
                            for icx in range(IC):
                                nc.tensor.matmul(agg_ps[h][icx][:], lhs,
                                                 p_t[:, 512 * icx:512 * (icx + 1)],
                                                 start=(jt == 0), stop=(jt == JT - 1))

                    evac = p3.enter_context(tc.tile_pool(name="evac", bufs=1))
                    raw_sb = evac.tile([D, NHL * ROWS], dt.bfloat16, tag="raw", name="raw")
                    zc_sb = evac.tile([1, NHL * IC * 512], dt.float32, tag="zc", name="zc")
                    for h in range(NHL):
                        for icx in range(IC):
                            k = h * IC + icx
                            nc.vector.tensor_copy(raw_sb[:, k * 512:(k + 1) * 512],
                                                  agg_ps[h][icx][0:D, :])
                            nc.scalar.copy(zc_sb[0:1, k * 512:(k + 1) * 512],
                                           agg_ps[h][icx][D:D + 1, :])
                    rrow = evac.tile([1, NHL * IC * 512], dt.float32, tag="rrow", name="rrow")
                    nc.vector.reciprocal(rrow[:], zc_sb[:])

                # ---- P4: h' = elu(raw / Z) + 1 = e^{min(x,0)} + relu(x) ----
                with ExitStack() as p4:
                    ps_rr = p4.enter_context(tc.tile_pool(name="psrr", bufs=2, space="PSUM"))
                    n4 = p4.enter_context(tc.tile_pool(name="n4", bufs=1))
                    rrep_sb = n4.tile([D, NHL * ROWS], dt.bfloat16, tag="rrep", name="rrep")
                    for k in range(NHL * IC):
                        rr_ps = ps_rr.tile([D, 512], dt.float32, tag="rrps", name="rrps")
                        nc.tensor.matmul(rr_ps[:], ones_sb[:, 0:D],
                                         rrow[:, 512 * k:512 * (k + 1)], start=True, stop=True)
                        nc.scalar.copy(rrep_sb[:, 512 * k:512 * (k + 1)], rr_ps[:])
                    t_n = n4.tile([D, NHL * ROWS], dt.bfloat16, tag="tn", name="tn")
                    nc.vector.tensor_tensor(t_n[:], raw_sb[:], rrep_sb[:], OP.mult)
                    m0 = n4.tile([D, NHL * ROWS], dt.bfloat16, tag="m0", name="m0")
                    nc.vector.tensor_scalar(m0[:], t_n[:], 0.0, None, op0=OP.min)
                    e0 = n4.tile([D, NHL * ROWS], dt.bfloat16, tag="e0", name="e0")
                    nc.scalar.activation(e0[:], m0[:], AF.Exp)
                    r0 = n4.tile([D, NHL * ROWS], dt.bfloat16, tag="r0", name="r0")
                    nc.vector.tensor_scalar(r0[:], t_n[:], 0.0, None, op0=OP.max)
                    hp1_sb = n4.tile([D, NHL * ROWS], dt.bfloat16, tag="hp1", name="hp1")
                    nc.vector.tensor_tensor(hp1_sb[:], e0[:], r0[:], OP.add)
                    for h in range(NHL):
                        for icx in range(IC):
                            o = (h * IC + icx) * 512
                            nc.sync.dma_start(
                                ht_shard.ap()[D * h:D * (h + 1), 512 * icx:512 * (icx + 1)],
                                hp1_sb[:, o:o + 512])

            # ---- P5: exchange (adjT2 prefetch overlaps collective + L2 prep) ----
            a2pf = l2pf.enter_context(tc.tile_pool(name="a2pf", bufs=1))
            adjT2_sb = a2pf.tile([128, 32 * 512], dt.bfloat16, tag="adjt2sb", name="adjt2sb")
            for _jt in range(32):
                nc.sync.dma_start(adjT2_sb[:, 512 * _jt:512 * (_jt + 1)],
                                  adjT2_in[128 * _jt:128 * (_jt + 1), :])
            nc.gpsimd.collective_compute(
                "AllGather", OP.bypass, replica_groups=[list(range(n_cores))],
                ins=[ht_shard.ap()], outs=[ht_all.ap()])

            # ---- P6: layer-2 prep ----
            with ExitStack() as p6:
                htp = p6.enter_context(tc.tile_pool(name="htp", bufs=1))
                l2p = p6.enter_context(tc.tile_pool(name="l2p", bufs=1))
                HT_sb = [htp.tile([128, N], dt.bfloat16, tag=f"ht{g}", name=f"ht{g}")
                         for g in range(OT)]
                for g in range(OT):
                    for rr in range(R):
                        core_idx = rr * HG + g
                        nc.sync.dma_start(HT_sb[g][:, ROWS * rr:ROWS * (rr + 1)],
                                          ht_all.ap()[DL * core_idx:DL * (core_idx + 1), :])
                Wo_sb = [l2p.tile([128, O], dt.bfloat16, tag=f"wo{g}", name=f"wo{g}")
                         for g in range(OT)]
                for g in range(OT):
                    nc.sync.dma_start(Wo_sb[g][:], wout_in[128 * g:128 * (g + 1), :])
                A2_sb = l2p.tile([O, 2], dt.bfloat16, tag="a2", name="a2")
                nc.sync.dma_start(A2_sb[:], a2_in)
                ncs_sb = l2p.tile([O, 1], dt.float32, tag="ncs", name="ncs")
                nc.sync.dma_start(ncs_sb[:], ncs_in)

                with ExitStack() as p6a:
                    p6w = p6a.enter_context(ExitStack())
                    ps_w2 = p6w.enter_context(tc.tile_pool(name="psw2", bufs=2, space="PSUM"))
                    ps_s2 = p6w.enter_context(tc.tile_pool(name="pss2", bufs=2, space="PSUM"))
                    Wh2T_sb = l2p.tile([O, N], dt.bfloat16, tag="wh2t", name="wh2t")
                    S2_sb = l2p.tile([2, N], dt.float32, tag="s2sb", name="s2sb")
                    for jc in range(JC):
                        w2_ps = ps_w2.tile([O, 512], dt.float32, tag="w2ps", name="w2ps")
                        for g in range(OT):
                            nc.tensor.matmul(w2_ps[:], Wo_sb[g][:],
                                             HT_sb[g][:, 512 * jc:512 * (jc + 1)],
                                             start=(g == 0), stop=(g == OT - 1))
                        # evacuate with the -colsum(W_out) correction (elu+1 shift)
                        nc.scalar.activation(Wh2T_sb[:, 512 * jc:512 * (jc + 1)], w2_ps[:],
                                             AF.Identity, bias=ncs_sb[:, 0:1], scale=1.0)
                        s2_ps = ps_s2.tile([2, 512], dt.float32, tag="s2ps", name="s2ps")
                        nc.tensor.matmul(s2_ps[:], A2_sb[:],
                                         Wh2T_sb[:, 512 * jc:512 * (jc + 1)],
                                         start=True, stop=True)
                        nc.scalar.copy(S2_sb[:, 512 * jc:512 * (jc + 1)], s2_ps[:])
                    nc.sync.dma_start(s1p_d.ap()[0:1, :], S2_sb[0:1, :])
                    nc.gpsimd.collective_compute(
                        "ReduceScatter", OP.add, replica_groups=[list(range(n_cores))],
                        ins=[s1p_d.ap()], outs=[s1p_rs.ap()])

                    p6w.close()
                    ps_t = p6a.enter_context(tc.tile_pool(name="pst", bufs=1, space="PSUM"))
                    ps_tr = p6a.enter_context(tc.tile_pool(name="pstr", bufs=2, space="PSUM"))
                    s2t_ps = ps_t.tile([128, 2 * JT], dt.float32, tag="s2tps", name="s2tps")
                    for jt in range(JT):
                        nc.tensor.matmul(s2t_ps[:, 2 * jt:2 * (jt + 1)],
                                         S2_sb[:, 128 * jt:128 * (jt + 1)],
                                         ident_sb[0:2, 0:2], is_transpose=True,
                                         start=True, stop=True)
                    S2T_sb = l2p.tile([128, 2 * JT], dt.float32, tag="s2tsb", name="s2tsb")
                    nc.vector.tensor_copy(S2T_sb[:], s2t_ps[:])

                    Wh2o_sb = l2p.tile([128, JT * (O + 2)], dt.bfloat16, tag="wh2o", name="wh2o")
                    w2o_view = Wh2o_sb[:].rearrange("p (j c) -> p j c", j=JT)
                    for jt in range(JT):
                        tr_ps = ps_tr.tile([128, 128], dt.bfloat16, tag="trps", name="trps")
                        nc.tensor.matmul(tr_ps[:], Wh2T_sb[:, 128 * jt:128 * (jt + 1)],
                                         identb_sb[:], is_transpose=True,
                                         start=True, stop=True)
                        nc.scalar.copy(w2o_view[:, jt, 0:O], tr_ps[:])
                    nc.vector.memset(w2o_view[:, :, O:O + 1], 1.0)

                    ps_r2 = p6a.enter_context(tc.tile_pool(name="psr2", bufs=2, space="PSUM"))
                    s1row2 = l2p.tile([1, ROWS2], dt.float32, tag="s1row2", name="s1row2")
                    nc.sync.dma_start(s1row2[:], s1p_rs.ap())
                    w1rep2_sb = l2p.tile([128, ROWS2], dt.bfloat16, tag="w1rep2", name="w1rep2")
                    for icx in range(IC2):
                        r2_ps = ps_r2.tile([128, 512], dt.float32, tag="r2ps", name="r2ps")
                        nc.tensor.matmul(r2_ps[:], ones_sb[:],
                                         s1row2[:, 512 * icx:512 * (icx + 1)],
                                         start=True, stop=True)
                        nc.scalar.activation(w1rep2_sb[:, 512 * icx:512 * (icx + 1)], r2_ps[:],
                                             AF.Exp, scale=(1.0 - SLOPE) / n_cores)
                    W2b_sb = l2p.tile([128, JT], dt.float32, tag="w2b", name="w2b")
                    V2b_sb = l2p.tile([128, JT], dt.float32, tag="v2b", name="v2b")
                    s2tv = S2T_sb[:].rearrange("p (j two) -> p j two", j=JT)
                    nc.scalar.activation(
                        W2b_sb[:].rearrange("p (j one) -> p j one", one=1),
                        s2tv[:, :, 1:2], AF.Exp, scale=1.0)
                    nc.scalar.activation(
                        V2b_sb[:].rearrange("p (j one) -> p j one", one=1),
                        s2tv[:, :, 1:2], AF.Exp, scale=SLOPE)

                # ---- P7: layer-2 attention ----
                with ExitStack() as p7:
                    t2p = p7.enter_context(tc.tile_pool(name="t2p", bufs=3))
                    p2p = p7.enter_context(tc.tile_pool(name="p2p", bufs=3))
                    agg2 = p7.enter_context(tc.tile_pool(name="agg2", bufs=1, space="PSUM"))
                    o_ps = [agg2.tile([128, O + 1], dt.float32, tag=f"ops{it}", name=f"ops{it}")
                            for it in range(ROWS2 // 128)]
                    for jt in range(JT):
                        mt2 = adjT2_sb[:, 512 * jt:512 * (jt + 1)]
                        t2t = t2p.tile([128, ROWS2], dt.bfloat16, tag="t2t", name="t2t")
                        nc.vector.tensor_scalar(t2t[:], w1rep2_sb[:],
                                                W2b_sb[:, jt:jt + 1], V2b_sb[:, jt:jt + 1],
                                                op0=OP.mult, op1=OP.max)
                        p2t = p2p.tile([128, ROWS2], dt.bfloat16, tag="p2t", name="p2t")
                        nc.vector.tensor_tensor(p2t[:], t2t[:], mt2, OP.mult)
                        for it in range(ROWS2 // 128):
                            nc.tensor.matmul(o_ps[it][:], p2t[:, 128 * it:128 * (it + 1)],
                                             w2o_view[:, jt, 0:O + 1],
                                             start=(jt == 0), stop=(jt == JT - 1))
                    fo = p7.enter_context(tc.tile_pool(name="fo", bufs=4))
                    for it in range(ROWS2 // 128):
                        r2v = fo.tile([128, 1], dt.float32, tag="r2v", name="r2v")
                        nc.vector.reciprocal(r2v[:], o_ps[it][:, O:O + 1])
                        ot = fo.tile([128, O], dt.float32, tag="ot", name="ot")
                        nc.vector.tensor_scalar(ot[:], o_ps[it][:, 0:O], r2v[:, 0:1], None,
                                                op0=OP.mult)
                        if reps == 1:
                            nc.sync.dma_start(out_ext[128 * it:128 * (it + 1), :], ot[:])
                        else:
                            # timing builds: accumulate so repeated bodies stay live
                            nc.gpsimd.dma_start(out_ext[128 * it:128 * (it + 1), :],
                                                ot[:], accum_op=OP.add)
            l2pf.close()

    nc.compile()
    return nc


def _make_in_maps(x, adj, W_heads, a1_heads, a2_heads, W_out, a1_out, a2_out,
                  n_cores=8, R=2):
    N, F = x.shape
    H, _, D = W_heads.shape
    HG = n_cores // R
    NHL = H // HG
    ROWS = N // R
    ROWS2 = N // n_cores

    xT = np.ascontiguousarray(x.T).astype(BF16)
    adjT = np.ascontiguousarray(adj.T).astype(BF16)
    ones = np.ones((1, 128), np.float32)
    ident = np.eye(128, dtype=np.float32)
    wout = np.ascontiguousarray(W_out).astype(BF16)
    a2m = np.stack([a1_out, a2_out], axis=1).astype(BF16)
    ncs = (-W_out.sum(axis=0)[:, None]).astype(np.float32)

    in_maps = []
    for c in range(n_cores):
        rr, hg = c // HG, c % HG
        heads = list(range(NHL * hg, NHL * (hg + 1)))
        r0 = ROWS * rr
        wloc = np.concatenate([W_heads[h] for h in heads], axis=1).astype(BF16)
        wtloc = np.concatenate([W_heads[h].T for h in heads], axis=0).astype(BF16)
        ablk = np.zeros((NHL * D, 2 * NHL), np.float32)
        for k, h in enumerate(heads):
            ablk[k * D:(k + 1) * D, 2 * k] = a1_heads[h]
            ablk[k * D:(k + 1) * D, 2 * k + 1] = a2_heads[h]
        in_maps.append({
            "xT": xT,
            "xTown": np.ascontiguousarray(xT[:, r0:r0 + ROWS]),
            "adjT": np.ascontiguousarray(adjT[:, r0:r0 + ROWS]),
            "adjT2": np.ascontiguousarray(adjT[:, ROWS2 * c:ROWS2 * (c + 1)]),
            "wloc": wloc,
            "wtloc": wtloc,
            "ablk": ablk.astype(BF16),
            "wout": wout,
            "a2": a2m,
            "ncs": ncs,
            "ones": ones,
            "ident": ident,
        })
    return in_maps


def kernel(x, adj, W_heads, a1_heads, a2_heads, W_out, a1_out, a2_out):
    x = np.asarray(x, dtype=np.float32)
    adj = np.asarray(adj)
    W_heads = np.asarray(W_heads, dtype=np.float32)
    a1_heads = np.asarray(a1_heads, dtype=np.float32)
    a2_heads = np.asarray(a2_heads, dtype=np.float32)
    W_out = np.asarray(W_out, dtype=np.float32)
    a1_out = np.asarray(a1_out, dtype=np.float32)
    a2_out = np.asarray(a2_out, dtype=np.float32)

    if "nc" not in _CACHE:
        _CACHE["nc"] = _build()
    nc = _CACHE["nc"]
    in_maps = _make_in_maps(x, adj, W_heads, a1_heads, a2_heads,
                            W_out, a1_out, a2_out)
    res = run_bass_kernel_spmd(nc, in_maps, list(range(8)))
    out = np.concatenate([r["out"] for r in res.results], axis=0)
    return out.astype(np.float32)


if __name__ == "__main__":
    import jax
    key = jax.random.key(0)
    ks = jax.random.split(key, 8)
    import jax.numpy as jnp
    N, F, D, H, O = 4096, 512, 64, 8, 128
    ins = {
        "x": np.asarray(jax.random.normal(ks[0], (N, F), dtype=jnp.float32)),
        "adj": np.asarray(jax.random.randint(ks[1], (N, N), 0, 2, dtype=jnp.int32)),
        "W_heads": np.asarray(jax.random.normal(ks[2], (H, F, D), dtype=jnp.float32) * 0.05),
        "a1_heads": np.asarray(jax.random.normal(ks[3], (H, D), dtype=jnp.float32) * 0.05),
        "a2_heads": np.asarray(jax.random.normal(ks[4], (H, D), dtype=jnp.float32) * 0.05),
        "W_out": np.asarray(jax.random.normal(ks[5], (H * D, O), dtype=jnp.float32) * 0.05),
        "a1_out": np.asarray(jax.random.normal(ks[6], (O, ), dtype=jnp.float32) * 0.05),
        "a2_out": np.asarray(jax.random.normal(ks[7], (O, ), dtype=jnp.float32) * 0.05),
    }
    out = kernel(**ins)
    print("out", out.shape, out.dtype, float(np.abs(out).max()))


# revision 31
# speedup vs baseline: 4.4810x; 3.4186x over previous
"""Trainium2 Bass kernel for a 2-layer dense GAT (nn_GAT_70446053589175).

kernel(**inputs) takes the FULL unsharded inputs (as produced by
setup_inputs) and returns the FULL [4096, 128] float32 output.

Sharding (8 NeuronCores, single SPMD program):
  Layer 1: 2 row-groups x 4 head-groups  (2048 rows, 2 heads per core)
  Layer 2: 8-way row split (512 rows per core)
  Exchange between layers via a single in-kernel AllGather collective; the
  per-core s1' slice for layer 2 is extracted with a one-hot selection
  matmul (every core computes the identical full s' row), so no
  ReduceScatter is needed.

Math (row-rescaled softmax numerator):
  softmax rows are invariant to per-row (i) positive scaling, so divide
  the numerator exp(leakyrelu_0.2(s1_i + s2_j)) by exp(0.2*s1_i):
    p'_ij = m_ij * max( e^{0.8 s1_i} * e^{s2_j},  e^{0.2 s2_j} )
  The leaky branch is now constant in i, so per (jt, head) tile:
    t  = tensor_scalar(w1rep, w2_j, v2_j, mult, max)   # DVE 4x mode
    p' = tensor_tensor(t, mask, mult)                  # DVE 2x mode
  (no per-tile ScalarE op, no 1x scalar_tensor_tensor).
  Aggregation on PE with lhsT = [Wh | 1]: row D accumulates Z.
  Layer-1 output is written as elu(h)+1 = e^{min(x,0)} + relu(x); the
  +1 shift is removed in layer 2 by subtracting colsum(W_out) from Wh2
  (softmax rows sum to 1), applied for free via the activation bias
  during PSUM evacuation.
Layout is [j on partitions, i on free] throughout so the softmax-weighted
aggregation contracts over partitions.
"""
import sys
import os

for _p in ("/opt/trn_rl_repo", "/opt/pypackages",
           os.path.expanduser("~/.axon_site/_ro/trn_rl_repo"),
           os.path.expanduser("~/.axon_site/_ro/pypackages")):
    if os.path.isdir(_p) and _p not in sys.path:
        sys.path.insert(0, _p)

from contextlib import ExitStack

import numpy as np
import ml_dtypes

import concourse.bacc as bacc
import concourse.tile as tile
from concourse import mybir
from concourse.bass_utils import run_bass_kernel_spmd

dt = mybir.dt
AF = mybir.ActivationFunctionType
OP = mybir.AluOpType

BF16 = ml_dtypes.bfloat16
SLOPE = 0.2
_CACHE = {}


def _build(N=4096, F=512, D=64, H=8, O=128, n_cores=8, R=2, reps=1):
    VAR = os.environ.get("KVARIANT", "full")
    NO_COLL = "nocoll" in VAR
    NJ_ACT = 20          # layer-1 jts on the ScalarE/fp8 path (paired)
    NJP = NJ_ACT // 2
    CEXP = 2.0           # exp bias offset for fp8 headroom (softmax-invariant)
    PADM = 80            # fp8 weight plane stride (multiple of 16)
    HG = n_cores // R
    NHL = H // HG
    ROWS = N // R
    ROWS2 = N // n_cores
    JT = N // 128
    FT = F // 128
    IC = ROWS // 512
    IC2 = ROWS2 // 512
    DL = NHL * D
    HD = H * D
    OT = HD // 128
    JC = N // 512
    assert DL == 128 and OT == HG and O == 128

    nc = bacc.Bacc("TRN2", target_bir_lowering=False, debug=False, num_devices=n_cores)

    xT_in = nc.dram_tensor("xT", [F, N], dt.bfloat16, kind="ExternalInput").ap()
    xTown_in = nc.dram_tensor("xTown", [F, ROWS], dt.bfloat16, kind="ExternalInput").ap()
    NJ_DVE = N // 128 - NJ_ACT
    adjTb_in = nc.dram_tensor("adjTb", [NJ_DVE * 128, ROWS], dt.bfloat16, kind="ExternalInput").ap()
    m16_in = nc.dram_tensor("m16", [NJP * 128, ROWS], dt.uint16, kind="ExternalInput").ap()
    adjT2_in = nc.dram_tensor("adjT2", [N, ROWS2], dt.bfloat16, kind="ExternalInput").ap()
    wloc_in = nc.dram_tensor("wloc", [F, DL], dt.bfloat16, kind="ExternalInput").ap()
    wtloc_in = nc.dram_tensor("wtloc", [DL, F], dt.bfloat16, kind="ExternalInput").ap()
    ablk_in = nc.dram_tensor("ablk", [DL, 2 * NHL], dt.bfloat16, kind="ExternalInput").ap()
    wout_in = nc.dram_tensor("wout", [HD, O], dt.bfloat16, kind="ExternalInput").ap()
    a2_in = nc.dram_tensor("a2", [O, 2], dt.bfloat16, kind="ExternalInput").ap()
    hot8_in = nc.dram_tensor("hot8", [8, 1], dt.float32, kind="ExternalInput").ap()
    ones_in = nc.dram_tensor("ones", [1, 128], dt.float32, kind="ExternalInput").ap()
    ident_in = nc.dram_tensor("ident", [128, 128], dt.float32, kind="ExternalInput").ap()
    out_ext = nc.dram_tensor("out", [ROWS2, O], dt.float32, kind="ExternalOutput").ap()

    ht_shard = nc.dram_tensor("ht_shard", [DL, ROWS], dt.bfloat16)
    ht_all = nc.dram_tensor("ht_all", [n_cores * DL, ROWS], dt.bfloat16, addr_space="Shared")
    s1p_d = nc.dram_tensor("s1p_d", [1, N], dt.float32)

    with tile.TileContext(nc) as tc, ExitStack() as top:
        cpool = top.enter_context(tc.tile_pool(name="const", bufs=1))
        ones_sb = cpool.tile([1, 128], dt.float32, tag="ones", name="ones")
        nc.sync.dma_start(ones_sb[:], ones_in)
        ident_sb = cpool.tile([128, 128], dt.float32, tag="ident", name="ident")
        nc.sync.dma_start(ident_sb[:], ident_in)
        identb_sb = cpool.tile([128, 128], dt.bfloat16, tag="identb", name="identb")
        nc.vector.tensor_copy(identb_sb[:], ident_sb[:])

        for _rep in range(reps):
            l2pf = ExitStack()
            with ExitStack() as l1s:
                l1pool = l1s.enter_context(tc.tile_pool(name="l1p", bufs=1))
                Wh_sb = l1pool.tile([128, NJ_DVE * NHL * (D + 1)], dt.bfloat16, tag="whsb", name="whsb")
                whp_sb = l1pool.tile([128, NJP * NHL * 2 * PADM], dt.float8e4, tag="whp", name="whp")
                S_sb = l1pool.tile([128, JT * 2 * NHL], dt.float32, tag="ssb", name="ssb")
                w1rep_sb = l1pool.tile([128, NHL * ROWS], dt.bfloat16, tag="w1rep", name="w1rep")
                s1rep_sb = l1pool.tile([128, NHL * ROWS], dt.bfloat16, tag="s1rep", name="s1rep")
                W2_sb = l1pool.tile([128, NHL * JT], dt.float32, tag="w2sb", name="w2sb")
                V2_sb = l1pool.tile([128, NHL * JT], dt.float32, tag="v2sb", name="v2sb")
                NS2_sb = l1pool.tile([128, NHL * JT], dt.float32, tag="ns2sb", name="ns2sb")
                B2_sb = l1pool.tile([128, NHL * JT], dt.float32, tag="b2sb", name="b2sb")

                # ---- P1a: Wh (all j, local heads) + S = [s1|s2] per head ----
                with ExitStack() as p1:
                    xt_pool = p1.enter_context(tc.tile_pool(name="xt", bufs=1))
                    w_pool = p1.enter_context(tc.tile_pool(name="wp", bufs=1))
                    wt_stack = p1.enter_context(ExitStack())
                    ps_wt = wt_stack.enter_context(tc.tile_pool(name="pswt", bufs=1, space="PSUM"))

                    xT_sb = [xt_pool.tile([128, N], dt.bfloat16, tag=f"xt{ft}", name=f"xt{ft}")
                             for ft in range(FT)]
                    xTo_sb = [xt_pool.tile([128, ROWS], dt.bfloat16, tag=f"xto{ft}", name=f"xto{ft}")
                              for ft in range(FT)]
                    W_sb = [w_pool.tile([128, DL], dt.bfloat16, tag=f"wl{ft}", name=f"wl{ft}")
                            for ft in range(FT)]
                    for ft in range(FT):
                        nc.sync.dma_start(xT_sb[ft][:], xT_in[128 * ft:128 * (ft + 1), :])
                        nc.sync.dma_start(xTo_sb[ft][:], xTown_in[128 * ft:128 * (ft + 1), :])
                        nc.sync.dma_start(W_sb[ft][:], wloc_in[128 * ft:128 * (ft + 1), :])
                    WT_sb = w_pool.tile([DL, F], dt.bfloat16, tag="wtl", name="wtl")
                    nc.sync.dma_start(WT_sb[:], wtloc_in)
                    A_sb = w_pool.tile([DL, 2 * NHL], dt.bfloat16, tag="ablk", name="ablk")
                    nc.sync.dma_start(A_sb[:], ablk_in)

                    wt_ps = ps_wt.tile([128, FT * 2 * NHL], dt.float32, tag="wtps", name="wtps")
                    for ft in range(FT):
                        nc.tensor.matmul(wt_ps[:, ft * 2 * NHL:(ft + 1) * 2 * NHL],
                                         WT_sb[:, 128 * ft:128 * (ft + 1)], A_sb[:],
                                         start=True, stop=True)
                    Wt_sb = w_pool.tile([128, FT * 2 * NHL], dt.bfloat16, tag="wtsb", name="wtsb")
                    nc.vector.tensor_copy(Wt_sb[:], wt_ps[:])
                    wt_stack.close()

                    # ---- P1b first: own-row s1 -> w1rep = exp(0.8 s1) ----
                    st_stack = p1.enter_context(ExitStack())
                    ps_st = st_stack.enter_context(tc.tile_pool(name="psst", bufs=1, space="PSUM"))
                    ps_rep = st_stack.enter_context(tc.tile_pool(name="psrep", bufs=2, space="PSUM"))
                    st_ps = ps_st.tile([2 * NHL, ROWS], dt.float32, tag="stps", name="stps")
                    for icx in range(IC):
                        for ft in range(FT):
                            nc.tensor.matmul(st_ps[:, 512 * icx:512 * (icx + 1)],
                                             Wt_sb[:, ft * 2 * NHL:(ft + 1) * 2 * NHL],
                                             xTo_sb[ft][:, 512 * icx:512 * (icx + 1)],
                                             start=(ft == 0), stop=(ft == FT - 1))
                    ST_sb = l1pool.tile([2 * NHL, ROWS], dt.float32, tag="stsb", name="stsb")
                    nc.vector.tensor_copy(ST_sb[:], st_ps[:])
                    s1row = [l1pool.tile([1, ROWS], dt.float32, tag=f"s1row{h}", name=f"s1row{h}")
                             for h in range(NHL)]
                    for h in range(NHL):
                        nc.sync.dma_start(s1row[h][:], ST_sb[2 * h:2 * h + 1, :])
                    for h in range(NHL):
                        for icx in range(IC):
                            rep_ps = ps_rep.tile([128, 512], dt.float32, tag="repps", name="repps")
                            nc.tensor.matmul(rep_ps[:], ones_sb[:],
                                             s1row[h][:, 512 * icx:512 * (icx + 1)],
                                             start=True, stop=True)
                            nc.scalar.copy(
                                s1rep_sb[:, h * ROWS + 512 * icx:h * ROWS + 512 * (icx + 1)],
                                rep_ps[:])
                    nc.scalar.activation(w1rep_sb[:], s1rep_sb[:], AF.Exp,
                                         scale=1.0 - SLOPE)
                    st_stack.close()
                    ps_whs = p1.enter_context(tc.tile_pool(name="pswhs", bufs=2, space="PSUM"))
                    ps_sall = p1.enter_context(tc.tile_pool(name="pssall", bufs=1, space="PSUM"))
                    s_all_ps = ps_sall.tile([128, JT * 2 * NHL], dt.float32,
                                            tag="sallps", name="sallps")

                    wh_view = Wh_sb[:].rearrange("p (j h d) -> p j h d", j=NJ_DVE, h=NHL)
                    nc.vector.memset(wh_view[:, :, :, D:D + 1], 1.0)
                    whp_view = whp_sb[:].rearrange("p (q h k m) -> p q h k m",
                                                   q=NJP, h=NHL, k=2)
                    nc.vector.memset(whp_view[:, :, :, :, D:D + 1], 1.0)
                    for jt in range(JT):
                        wh_ps = ps_whs.tile([128, DL], dt.float32, tag="whps", name="whps")
                        for ft in range(FT):
                            lhs = xT_sb[ft][:, 128 * jt:128 * (jt + 1)]
                            nc.tensor.matmul(wh_ps[:], lhs, W_sb[ft][:],
                                             start=(ft == 0), stop=(ft == FT - 1))
                            nc.tensor.matmul(s_all_ps[:, jt * 2 * NHL:(jt + 1) * 2 * NHL],
                                             lhs,
                                             Wt_sb[:, ft * 2 * NHL:(ft + 1) * 2 * NHL],
                                             start=(ft == 0), stop=(ft == FT - 1))
                        src = wh_ps[:].rearrange("p (h d) -> p h d", h=NHL)
                        if jt < NJ_ACT:
                            nc.scalar.copy(whp_view[:, jt // 2, :, jt % 2, 0:D], src)
                        else:
                            nc.scalar.copy(wh_view[:, jt - NJ_ACT, :, 0:D], src)
                    nc.vector.tensor_copy(S_sb[:], s_all_ps[:])
                    # batched per-(h,jt) scalars: w2 = e^{s2}, v2 = e^{0.2 s2}
                    sl = S_sb[:].rearrange("p (j h two) -> p h j two", j=JT, h=NHL)
                    w2v = W2_sb[:].rearrange("p (h j) -> p h j", h=NHL)
                    v2v = V2_sb[:].rearrange("p (h j) -> p h j", h=NHL)
                    ns2v = NS2_sb[:].rearrange("p (h j) -> p h j", h=NHL)
                    b2v = B2_sb[:].rearrange("p (h j) -> p h j", h=NHL)
                    cm2_sb = l1pool.tile([128, 1], dt.float32, tag="cm2", name="cm2")
                    nc.vector.memset(cm2_sb[:], -CEXP)
                    for h in range(NHL):
                        nc.scalar.activation(
                            w2v[:, h:h + 1, :].rearrange("p one j -> p j one"),
                            sl[:, h, :, 1:2], AF.Exp, scale=1.0, bias=cm2_sb[:, 0:1])
                        nc.scalar.activation(
                            v2v[:, h:h + 1, :].rearrange("p one j -> p j one"),
                            sl[:, h, :, 1:2], AF.Exp, scale=SLOPE, bias=cm2_sb[:, 0:1])
                        nc.vector.tensor_scalar(
                            ns2v[:, h:h + 1, :].rearrange("p one j -> p j one"),
                            sl[:, h, :, 1:2], -1.0, None, op0=OP.mult)
                        nc.vector.tensor_scalar(
                            b2v[:, h:h + 1, :].rearrange("p one j -> p j one"),
                            sl[:, h, :, 1:2], -CEXP, None, op0=OP.add)

                # ---- P3: layer-1 attention + aggregation (two paths) ----
                with ExitStack() as p3:
                    m16pool = p3.enter_context(tc.tile_pool(name="m16p", bufs=4))
                    mtpool = p3.enter_context(tc.tile_pool(name="mtp", bufs=4))
                    gpool = p3.enter_context(tc.tile_pool(name="gtp", bufs=4))
                    pkpool = p3.enter_context(tc.tile_pool(name="pkp", bufs=4))
                    pmpool = p3.enter_context(tc.tile_pool(name="pmp", bufs=4))
                    tpool = p3.enter_context(tc.tile_pool(name="tpl", bufs=4))
                    ppool = p3.enter_context(tc.tile_pool(name="ppl", bufs=4))
                    agg = p3.enter_context(tc.tile_pool(name="agg", bufs=1, space="PSUM"))
                    agg_ps = [[agg.tile([D + 1, 512], dt.float32, tag=f"agg{h}_{icx}",
                                        name=f"agg{h}_{icx}")
                               for icx in range(IC)] for h in range(NHL)]

                    # Interleave ScalarE/fp8 pairs with DVE/bf16 jts so both
                    # engines stay busy (pair 0 first: it carries PSUM start).
                    def emit_pair(pp):
                        m16t = m16pool.tile([128, ROWS], dt.uint16, tag="m16", name="m16")
                        nc.sync.dma_start(m16t[:], m16_in[128 * pp:128 * (pp + 1), :])
                        for h in range(NHL):
                            ppk = pkpool.tile([128, 2 * ROWS], dt.float8e4, tag="ppk", name="ppk")
                            for ko in range(2):
                                jt = 2 * pp + ko
                                gt = gpool.tile([128, ROWS], dt.bfloat16, tag="gt", name="gt")
                                nc.vector.tensor_scalar(
                                    gt[:], s1rep_sb[:, h * ROWS:(h + 1) * ROWS],
                                    NS2_sb[:, h * JT + jt:h * JT + jt + 1], None, op0=OP.max)
                                nc.scalar.activation(
                                    ppk[:, ko * ROWS:(ko + 1) * ROWS], gt[:], AF.Exp,
                                    scale=1.0 - SLOPE,
                                    bias=B2_sb[:, h * JT + jt:h * JT + jt + 1])
                            pm = pmpool.tile([128, 2 * ROWS], dt.float8e4, tag="pm", name="pm")
                            nc.vector.tensor_tensor(pm[:].bitcast(dt.uint16),
                                                    ppk[:].bitcast(dt.uint16),
                                                    m16t[:], OP.bitwise_and)
                            lhsp = whp_view[:, pp, h, :, 0:D + 1]
                            rhs_v = pm[:].rearrange("p (k i) -> p k i", k=2)
                            for icx in range(IC):
                                nc.tensor.matmul(agg_ps[h][icx][:], lhsp,
                                                 rhs_v[:, :, 512 * icx:512 * (icx + 1)],
                                                 start=(pp == 0), stop=False,
                                                 perf_mode=mybir.MatmulPerfMode.DoubleRow)

                    def emit_dve(dk):
                        jt = NJ_ACT + dk
                        mt = mtpool.tile([128, ROWS], dt.bfloat16, tag="mt", name="mt")
                        nc.sync.dma_start(mt[:], adjTb_in[128 * dk:128 * (dk + 1), :])
                        for h in range(NHL):
                            t_t = tpool.tile([128, ROWS], dt.bfloat16, tag="t", name="t")
                            nc.vector.tensor_scalar(
                                t_t[:], w1rep_sb[:, h * ROWS:(h + 1) * ROWS],
                                W2_sb[:, h * JT + jt:h * JT + jt + 1],
                                V2_sb[:, h * JT + jt:h * JT + jt + 1],
                                op0=OP.mult, op1=OP.max)
                            p_tt = ppool.tile([128, ROWS], dt.bfloat16, tag="p", name="p")
                            nc.vector.tensor_tensor(p_tt[:], t_t[:], mt[:], OP.mult)
                            lhs = wh_view[:, dk, h, 0:D + 1]
                            for icx in range(IC):
                                nc.tensor.matmul(agg_ps[h][icx][:], lhs,
                                                 p_tt[:, 512 * icx:512 * (icx + 1)],
                                                 start=False, stop=(dk == NJ_DVE - 1))

                    emit_pair(0)
                    pi, di = 1, 0
                    while pi < NJP or di < NJ_DVE:
                        if pi < NJP:
                            emit_pair(pi)
                            pi += 1
                        if di < NJ_DVE:
                            emit_dve(di)
                            di += 1

                    evac = p3.enter_context(tc.tile_pool(name="evac", bufs=1))
                    raw_sb = evac.tile([D, NHL * ROWS], dt.bfloat16, tag="raw", name="raw")
                    zc_sb = evac.tile([1, NHL * IC * 512], dt.float32, tag="zc", name="zc")
                    for h in range(NHL):
                        for icx in range(IC):
                            k = h * IC + icx
                            nc.vector.tensor_copy(raw_sb[:, k * 512:(k + 1) * 512],
                                                  agg_ps[h][icx][0:D, :])
                            nc.scalar.copy(zc_sb[0:1, k * 512:(k + 1) * 512],
                                           agg_ps[h][icx][D:D + 1, :])
                    rrow = evac.tile([1, NHL * IC * 512], dt.float32, tag="rrow", name="rrow")
                    nc.vector.reciprocal(rrow[:], zc_sb[:])

                # ---- P4: h' = elu(raw / Z) + 1 = e^{min(x,0)} + relu(x) ----
                with ExitStack() as p4:
                    ps_rr = p4.enter_context(tc.tile_pool(name="psrr", bufs=2, space="PSUM"))
                    n4 = p4.enter_context(tc.tile_pool(name="n4", bufs=1))
                    rrep_sb = n4.tile([D, NHL * ROWS], dt.bfloat16, tag="rrep", name="rrep")
                    for k in range(NHL * IC):
                        rr_ps = ps_rr.tile([D, 512], dt.float32, tag="rrps", name="rrps")
                        nc.tensor.matmul(rr_ps[:], ones_sb[:, 0:D],
                                         rrow[:, 512 * k:512 * (k + 1)], start=True, stop=True)
                        nc.scalar.copy(rrep_sb[:, 512 * k:512 * (k + 1)], rr_ps[:])
                    t_n = n4.tile([D, NHL * ROWS], dt.bfloat16, tag="tn", name="tn")
                    nc.vector.tensor_tensor(t_n[:], raw_sb[:], rrep_sb[:], OP.mult)
                    m0 = n4.tile([D, NHL * ROWS], dt.bfloat16, tag="m0", name="m0")
                    nc.vector.tensor_scalar(m0[:], t_n[:], 0.0, None, op0=OP.min)
                    e0 = n4.tile([D, NHL * ROWS], dt.bfloat16, tag="e0", name="e0")
                    nc.scalar.activation(e0[:], m0[:], AF.Exp)
                    r0 = n4.tile([D, NHL * ROWS], dt.bfloat16, tag="r0", name="r0")
                    nc.vector.tensor_scalar(r0[:], t_n[:], 0.0, None, op0=OP.max)
                    e0m1 = n4.tile([D, NHL * ROWS], dt.bfloat16, tag="e0m1", name="e0m1")
                    nc.vector.tensor_scalar(e0m1[:], e0[:], 1.0, None, op0=OP.subtract)
                    hp1_sb = n4.tile([D, NHL * ROWS], dt.bfloat16, tag="hp1", name="hp1")
                    nc.vector.tensor_tensor(hp1_sb[:], e0m1[:], r0[:], OP.add)
                    for h in range(NHL):
                        for icx in range(IC):
                            o = (h * IC + icx) * 512
                            nc.sync.dma_start(
                                ht_shard.ap()[D * h:D * (h + 1), 512 * icx:512 * (icx + 1)],
                                hp1_sb[:, o:o + 512])

            # ---- P5: exchange (adjT2 prefetch overlaps collective + L2 prep) ----
            a2pf = l2pf.enter_context(tc.tile_pool(name="a2pf", bufs=1))
            adjT2_sb = a2pf.tile([128, 32 * 512], dt.bfloat16, tag="adjt2sb", name="adjt2sb")
            for _jt in range(32):
                nc.sync.dma_start(adjT2_sb[:, 512 * _jt:512 * (_jt + 1)],
                                  adjT2_in[128 * _jt:128 * (_jt + 1), :])
            if NO_COLL:
                for g in range(n_cores):
                    nc.sync.dma_start(ht_all.ap()[DL * g:DL * (g + 1), :], ht_shard.ap())
            else:
                nc.gpsimd.collective_compute(
                    "AllGather", OP.bypass, replica_groups=[list(range(n_cores))],
                    ins=[ht_shard.ap()], outs=[ht_all.ap()])

            # ---- P6: layer-2 prep ----
            with ExitStack() as p6:
                htp = p6.enter_context(tc.tile_pool(name="htp", bufs=1))
                l2p = p6.enter_context(tc.tile_pool(name="l2p", bufs=1))
                HT_sb = [htp.tile([128, N], dt.bfloat16, tag=f"ht{g}", name=f"ht{g}")
                         for g in range(OT)]
                for g in range(OT):
                    for rr in range(R):
                        core_idx = rr * HG + g
                        nc.sync.dma_start(HT_sb[g][:, ROWS * rr:ROWS * (rr + 1)],
                                          ht_all.ap()[DL * core_idx:DL * (core_idx + 1), :])
                Wo_sb = [l2p.tile([128, O], dt.bfloat16, tag=f"wo{g}", name=f"wo{g}")
                         for g in range(OT)]
                for g in range(OT):
                    nc.sync.dma_start(Wo_sb[g][:], wout_in[128 * g:128 * (g + 1), :])
                A2_sb = l2p.tile([O, 2], dt.bfloat16, tag="a2", name="a2")
                nc.sync.dma_start(A2_sb[:], a2_in)

                with ExitStack() as p6a:
                    p6w = p6a.enter_context(ExitStack())
                    ps_w2 = p6w.enter_context(tc.tile_pool(name="psw2", bufs=2, space="PSUM"))
                    ps_s2 = p6w.enter_context(tc.tile_pool(name="pss2", bufs=2, space="PSUM"))
                    Wh2T_sb = l2p.tile([O, N], dt.bfloat16, tag="wh2t", name="wh2t")
                    S2_sb = l2p.tile([2, N], dt.float32, tag="s2sb", name="s2sb")
                    for jc in range(JC):
                        w2_ps = ps_w2.tile([O, 512], dt.float32, tag="w2ps", name="w2ps")
                        for g in range(OT):
                            nc.tensor.matmul(w2_ps[:], Wo_sb[g][:],
                                             HT_sb[g][:, 512 * jc:512 * (jc + 1)],
                                             start=(g == 0), stop=(g == OT - 1))
                        nc.scalar.copy(Wh2T_sb[:, 512 * jc:512 * (jc + 1)], w2_ps[:])
                        s2_ps = ps_s2.tile([2, 512], dt.float32, tag="s2ps", name="s2ps")
                        nc.tensor.matmul(s2_ps[:], A2_sb[:],
                                         Wh2T_sb[:, 512 * jc:512 * (jc + 1)],
                                         start=True, stop=True)
                        nc.scalar.copy(S2_sb[:, 512 * jc:512 * (jc + 1)], s2_ps[:])
                    # per-core s1' slice via one-hot selection (no collective):
                    # bounce the s1' row through DRAM to reshape [1,4096]->[8,512]
                    nc.sync.dma_start(s1p_d.ap()[0:1, :], S2_sb[0:1, :])
                    s1p8 = l2p.tile([8, 512], dt.float32, tag="s1p8", name="s1p8")
                    nc.sync.dma_start(
                        s1p8[:], s1p_d.ap()[0:1, :].rearrange("one (p f) -> (one p) f", p=8))
                    hot8_sb = l2p.tile([8, 1], dt.float32, tag="hot8", name="hot8")
                    nc.sync.dma_start(hot8_sb[:], hot8_in)

                    p6w.close()
                    ps_t = p6a.enter_context(tc.tile_pool(name="pst", bufs=1, space="PSUM"))
                    ps_tr = p6a.enter_context(tc.tile_pool(name="pstr", bufs=2, space="PSUM"))
                    s2t_ps = ps_t.tile([128, 2 * JT], dt.float32, tag="s2tps", name="s2tps")
                    for jt in range(JT):
                        nc.tensor.matmul(s2t_ps[:, 2 * jt:2 * (jt + 1)],
                                         S2_sb[:, 128 * jt:128 * (jt + 1)],
                                         ident_sb[0:2, 0:2], is_transpose=True,
                                         start=True, stop=True)
                    S2T_sb = l2p.tile([128, 2 * JT], dt.float32, tag="s2tsb", name="s2tsb")
                    nc.vector.tensor_copy(S2T_sb[:], s2t_ps[:])

                    Wh2o_sb = l2p.tile([128, JT * (O + 2)], dt.bfloat16, tag="wh2o", name="wh2o")
                    w2o_view = Wh2o_sb[:].rearrange("p (j c) -> p j c", j=JT)
                    for jt in range(JT):
                        tr_ps = ps_tr.tile([128, 128], dt.bfloat16, tag="trps", name="trps")
                        nc.tensor.matmul(tr_ps[:], Wh2T_sb[:, 128 * jt:128 * (jt + 1)],
                                         identb_sb[:], is_transpose=True,
                                         start=True, stop=True)
                        nc.scalar.copy(w2o_view[:, jt, 0:O], tr_ps[:])
                    nc.vector.memset(w2o_view[:, :, O:O + 1], 1.0)

                    ps_r2 = p6a.enter_context(tc.tile_pool(name="psr2", bufs=2, space="PSUM"))
                    sel_ps = ps_r2.tile([1, ROWS2], dt.float32, tag="selps", name="selps")
                    nc.tensor.matmul(sel_ps[:], hot8_sb[:], s1p8[:], start=True, stop=True)
                    s1row2 = l2p.tile([1, ROWS2], dt.float32, tag="s1row2", name="s1row2")
                    nc.scalar.copy(s1row2[:], sel_ps[:])
                    w1rep2_sb = l2p.tile([128, ROWS2], dt.bfloat16, tag="w1rep2", name="w1rep2")
                    for icx in range(IC2):
                        r2_ps = ps_r2.tile([128, 512], dt.float32, tag="r2ps", name="r2ps")
                        nc.tensor.matmul(r2_ps[:], ones_sb[:],
                                         s1row2[:, 512 * icx:512 * (icx + 1)],
                                         start=True, stop=True)
                        nc.scalar.activation(w1rep2_sb[:, 512 * icx:512 * (icx + 1)], r2_ps[:],
                                             AF.Exp, scale=(1.0 - SLOPE))
                    W2b_sb = l2p.tile([128, JT], dt.float32, tag="w2b", name="w2b")
                    V2b_sb = l2p.tile([128, JT], dt.float32, tag="v2b", name="v2b")
                    s2tv = S2T_sb[:].rearrange("p (j two) -> p j two", j=JT)
                    nc.scalar.activation(
                        W2b_sb[:].rearrange("p (j one) -> p j one", one=1),
                        s2tv[:, :, 1:2], AF.Exp, scale=1.0)
                    nc.scalar.activation(
                        V2b_sb[:].rearrange("p (j one) -> p j one", one=1),
                        s2tv[:, :, 1:2], AF.Exp, scale=SLOPE)

                # ---- P7: layer-2 attention ----
                with ExitStack() as p7:
                    t2p = p7.enter_context(tc.tile_pool(name="t2p", bufs=3))
                    p2p = p7.enter_context(tc.tile_pool(name="p2p", bufs=3))
                    agg2 = p7.enter_context(tc.tile_pool(name="agg2", bufs=1, space="PSUM"))
                    o_ps = [agg2.tile([128, O + 1], dt.float32, tag=f"ops{it}", name=f"ops{it}")
                            for it in range(ROWS2 // 128)]
                    for jt in range(JT):
                        mt2 = adjT2_sb[:, 512 * jt:512 * (jt + 1)]
                        t2t = t2p.tile([128, ROWS2], dt.bfloat16, tag="t2t", name="t2t")
                        nc.vector.tensor_scalar(t2t[:], w1rep2_sb[:],
                                                W2b_sb[:, jt:jt + 1], V2b_sb[:, jt:jt + 1],
                                                op0=OP.mult, op1=OP.max)
                        p2t = p2p.tile([128, ROWS2], dt.bfloat16, tag="p2t", name="p2t")
                        nc.vector.tensor_tensor(p2t[:], t2t[:], mt2, OP.mult)
                        p2v = p2t[:]
                        for it in range(ROWS2 // 128):
                            nc.tensor.matmul(o_ps[it][:], p2v[:, 128 * it:128 * (it + 1)],
                                             w2o_view[:, jt, 0:O + 1],
                                             start=(jt == 0), stop=(jt == JT - 1))
                    fo = p7.enter_context(tc.tile_pool(name="fo", bufs=4))
                    for it in range(ROWS2 // 128):
                        r2v = fo.tile([128, 1], dt.float32, tag="r2v", name="r2v")
                        nc.vector.reciprocal(r2v[:], o_ps[it][:, O:O + 1])
                        ot = fo.tile([128, O], dt.float32, tag="ot", name="ot")
                        nc.vector.tensor_scalar(ot[:], o_ps[it][:, 0:O], r2v[:, 0:1], None,
                                                op0=OP.mult)
                        if reps == 1:
                            nc.sync.dma_start(out_ext[128 * it:128 * (it + 1), :], ot[:])
                        else:
                            # timing builds: accumulate so repeated bodies stay live
                            nc.gpsimd.dma_start(out_ext[128 * it:128 * (it + 1), :],
                                                ot[:], accum_op=OP.add)
            l2pf.close()

    nc.compile()
    return nc


def _make_in_maps(x, adj, W_heads, a1_heads, a2_heads, W_out, a1_out, a2_out,
                  n_cores=8, R=2):
    N, F = x.shape
    H, _, D = W_heads.shape
    HG = n_cores // R
    NHL = H // HG
    ROWS = N // R
    ROWS2 = N // n_cores

    NJ_ACT, NJP = 20, 10
    xT = np.ascontiguousarray(x.T).astype(BF16)
    adjT = np.ascontiguousarray(adj.T).astype(BF16)
    adjTu8 = (np.ascontiguousarray(adj.T) > 0).astype(np.uint8) * np.uint8(0xFF)
    ones = np.ones((1, 128), np.float32)
    ident = np.eye(128, dtype=np.float32)
    wout = np.ascontiguousarray(W_out).astype(BF16)
    a2m = np.stack([a1_out, a2_out], axis=1).astype(BF16)

    in_maps = []
    for c in range(n_cores):
        rr, hg = c // HG, c % HG
        heads = list(range(NHL * hg, NHL * (hg + 1)))
        r0 = ROWS * rr
        wloc = np.concatenate([W_heads[h] for h in heads], axis=1).astype(BF16)
        wtloc = np.concatenate([W_heads[h].T for h in heads], axis=0).astype(BF16)
        ablk = np.zeros((NHL * D, 2 * NHL), np.float32)
        for k, h in enumerate(heads):
            ablk[k * D:(k + 1) * D, 2 * k] = a1_heads[h]
            ablk[k * D:(k + 1) * D, 2 * k + 1] = a2_heads[h]
        hot8 = np.zeros((8, 1), np.float32)
        hot8[c, 0] = 1.0
        adjTbc = np.concatenate(
            [adjT[128 * jt:128 * (jt + 1), r0:r0 + ROWS]
             for jt in range(NJ_ACT, N // 128)], axis=0)
        m16c = np.concatenate(
            [np.ascontiguousarray(np.concatenate(
                [adjTu8[128 * (2 * q):128 * (2 * q + 1), r0:r0 + ROWS],
                 adjTu8[128 * (2 * q + 1):128 * (2 * q + 2), r0:r0 + ROWS]],
                axis=1)).view(np.uint16)
             for q in range(NJP)], axis=0)
        in_maps.append({
            "xT": xT,
            "xTown": np.ascontiguousarray(xT[:, r0:r0 + ROWS]),
            "adjTb": np.ascontiguousarray(adjTbc),
            "m16": m16c,
            "adjT2": np.ascontiguousarray(adjT[:, ROWS2 * c:ROWS2 * (c + 1)]),
            "wloc": wloc,
            "wtloc": wtloc,
            "ablk": ablk.astype(BF16),
            "wout": wout,
            "a2": a2m,
            "hot8": hot8,
            "ones": ones,
            "ident": ident,
        })
    return in_maps


def kernel(x, adj, W_heads, a1_heads, a2_heads, W_out, a1_out, a2_out):
    x = np.asarray(x, dtype=np.float32)
    adj = np.asarray(adj)
    W_heads = np.asarray(W_heads, dtype=np.float32)
    a1_heads = np.asarray(a1_heads, dtype=np.float32)
    a2_heads = np.asarray(a2_heads, dtype=np.float32)
    W_out = np.asarray(W_out, dtype=np.float32)
    a1_out = np.asarray(a1_out, dtype=np.float32)
    a2_out = np.asarray(a2_out, dtype=np.float32)

    if "nc" not in _CACHE:
        _CACHE["nc"] = _build()
    nc = _CACHE["nc"]
    in_maps = _make_in_maps(x, adj, W_heads, a1_heads, a2_heads,
                            W_out, a1_out, a2_out)
    res = run_bass_kernel_spmd(nc, in_maps, list(range(8)))
    out = np.concatenate([r["out"] for r in res.results], axis=0)
    return out.astype(np.float32)


if __name__ == "__main__":
    import jax
    key = jax.random.key(0)
    ks = jax.random.split(key, 8)
    import jax.numpy as jnp
    N, F, D, H, O = 4096, 512, 64, 8, 128
    ins = {
        "x": np.asarray(jax.random.normal(ks[0], (N, F), dtype=jnp.float32)),
        "adj": np.asarray(jax.random.randint(ks[1], (N, N), 0, 2, dtype=jnp.int32)),
        "W_heads": np.asarray(jax.random.normal(ks[2], (H, F, D), dtype=jnp.float32) * 0.05),
        "a1_heads": np.asarray(jax.random.normal(ks[3], (H, D), dtype=jnp.float32) * 0.05),
        "a2_heads": np.asarray(jax.random.normal(ks[4], (H, D), dtype=jnp.float32) * 0.05),
        "W_out": np.asarray(jax.random.normal(ks[5], (H * D, O), dtype=jnp.float32) * 0.05),
        "a1_out": np.asarray(jax.random.normal(ks[6], (O, ), dtype=jnp.float32) * 0.05),
        "a2_out": np.asarray(jax.random.normal(ks[7], (O, ), dtype=jnp.float32) * 0.05),
    }
    out = kernel(**ins)
    print("out", out.shape, out.dtype, float(np.abs(out).max()))
